# revision 2
# baseline (speedup 1.0000x reference)
"""Multi-head attention (B=2, S=2048, D=1024, H=16) on 8 NeuronCores.

Sharding: core c -> batch c//4, head-group c%4 (4 heads, 256 proj dims).
Per-core Bass/Tile kernel computes Q/K/V projections, transposed-scores
attention (k on partitions, softmax without max-subtraction), and a
partial output projection (row-parallel Wo). Host sums the 4 partials
per batch and adds bo.

All matmuls run as float32r (TF32-like, 1 cycle/row) with fp32 PSUM
accumulation; data is shipped as fp32 bits (float32r is bit-compatible).
"""

import sys

sys.path.insert(0, "/opt/trn_rl_repo")

from contextlib import ExitStack

import numpy as np

import concourse.bacc as bacc
import concourse.mybir as mybir
import concourse.tile as tile
from concourse.bass_utils import run_bass_kernel_spmd

B = 2
S = 2048
D = 1024
H = 16
HD = 64
HPC = 4          # heads per core
DPC = HPC * HD   # 256 projection dims per core
NCORES = 8
SCALE = 8.0      # sqrt(HD)

F32 = mybir.dt.float32
F32R = mybir.dt.float32r
BF16 = mybir.dt.bfloat16
ADT = BF16   # attention operand dtype (qt/kt/v/pt)

DCH = D // 128   # 8 contraction chunks of 128
QT = S // 128    # 16 q-tiles / k-tiles of 128
QCN = 2          # attention q-chunks of 1024
QCW = 1024


def build_nc():
    nc = bacc.Bacc("TRN2", target_bir_lowering=False, debug=False, num_devices=NCORES)

    xq = nc.dram_tensor("xq_t", [D, S], F32R, kind="ExternalInput")
    xk = nc.dram_tensor("xk_t", [D, S], F32R, kind="ExternalInput")
    xv = nc.dram_tensor("xv_t", [D, S], F32R, kind="ExternalInput")
    wq = nc.dram_tensor("wq_t", [D, DPC], F32R, kind="ExternalInput")
    wk = nc.dram_tensor("wk_t", [D, DPC], F32R, kind="ExternalInput")
    wv = nc.dram_tensor("wv_t", [D, DPC], F32R, kind="ExternalInput")
    wo = nc.dram_tensor("wo_t", [DPC, D], F32R, kind="ExternalInput")
    bq = nc.dram_tensor("bq", [DPC, 1], F32, kind="ExternalInput")
    bk = nc.dram_tensor("bk", [DPC, 1], F32, kind="ExternalInput")
    bv = nc.dram_tensor("bv", [DPC, 1], F32, kind="ExternalInput")
    ident = nc.dram_tensor("ident", [128, 128], F32R, kind="ExternalInput")
    y = nc.dram_tensor("y", [S, D], F32, kind="ExternalOutput")

    with tile.TileContext(nc) as tc, ExitStack() as ctx:
        const = ctx.enter_context(tc.tile_pool(name="const", bufs=1))
        xin = ctx.enter_context(tc.tile_pool(name="xin", bufs=5))
        qkv = ctx.enter_context(tc.tile_pool(name="qkv", bufs=1))
        ptp = ctx.enter_context(tc.tile_pool(name="ptp", bufs=2))
        nrm = ctx.enter_context(tc.tile_pool(name="nrm", bufs=2))
        yp = ctx.enter_context(tc.tile_pool(name="yp", bufs=3))
        # tag semantics: same tag -> rotate through `bufs` slots;
        # distinct tags -> independent allocations.

        # ---- constants / weights ----
        # tiny dummy exp first: preloads the ACT exp table set off the
        # critical path (a ~4us PE-idle gap at attention start re-throttles
        # the PE clock to 1.2GHz for the rest of the kernel otherwise)
        dmy = const.tile([1, 16], F32, tag="dmy")
        nc.vector.memset(dmy[:], 0.0)
        dmy2 = const.tile([1, 16], F32, tag="dmy2")
        nc.scalar.activation(dmy2[:], dmy[:], mybir.ActivationFunctionType.Exp)

        # memset can't target f32r; stage in f32 and round via DVE copy
        ones32 = const.tile([1, 128], F32, tag="ones32")
        nc.vector.memset(ones32[:], 1.0)
        ones = const.tile([1, 128], F32R, tag="ones")
        nc.vector.tensor_copy(ones[:], ones32[:])
        onesv32 = const.tile([128, HPC], F32, tag="onesv32")
        nc.vector.memset(onesv32[:], 1.0)
        onesv = const.tile([128, HPC], F32R, tag="onesv")
        nc.vector.tensor_copy(onesv[:], onesv32[:])
        wq_sb = [const.tile([128, DPC], F32R, tag=f"wq{d}", name=f"wq{d}") for d in range(DCH)]
        wk_sb = [const.tile([128, DPC], F32R, tag=f"wk{d}", name=f"wk{d}") for d in range(DCH)]
        wv_sb = [const.tile([128, DPC], F32R, tag=f"wv{d}", name=f"wv{d}") for d in range(DCH)]
        for d in range(DCH):
            nc.sync.dma_start(wq_sb[d][:], wq[d * 128:(d + 1) * 128, :])
            nc.sync.dma_start(wk_sb[d][:], wk[d * 128:(d + 1) * 128, :])
            nc.sync.dma_start(wv_sb[d][:], wv[d * 128:(d + 1) * 128, :])
        wo_sb = [const.tile([128, D], F32R, tag=f"wo{g}", name=f"wo{g}") for g in range(2)]
        for g in range(2):
            nc.sync.dma_start(wo_sb[g][:], wo[g * 128:(g + 1) * 128, :])
        bq_sb = [const.tile([128, 1], F32, tag=f"bq{hp}", name=f"bq{hp}") for hp in range(2)]
        bk_sb = [const.tile([128, 1], F32, tag=f"bk{hp}", name=f"bk{hp}") for hp in range(2)]
        bv_sb = [const.tile([128, 1], F32, tag=f"bv{hp}", name=f"bv{hp}") for hp in range(2)]
        for hp in range(2):
            nc.sync.dma_start(bq_sb[hp][:], bq[hp * 128:(hp + 1) * 128, :])
            nc.sync.dma_start(bk_sb[hp][:], bk[hp * 128:(hp + 1) * 128, :])
            nc.sync.dma_start(bv_sb[hp][:], bv[hp * 128:(hp + 1) * 128, :])
        id_sb = const.tile([128, 128], F32R, tag="id")
        nc.sync.dma_start(id_sb[:], ident[:])

        # ---- V tiles (128, 4*65) with ones column, filled by PE transpose
        # of a V.T projection (weight-stationary like Q/K; avoids the
        # per-matmul LDWEIGHTS serialization of an x-stationary V-proj) ----
        v_sb = [qkv.tile([128, HPC * (HD + 1)], ADT, tag=f"v{st}", name=f"v{st}") for st in range(QT)]
        for st in range(QT):
            v4 = v_sb[st][:].rearrange("p (h w) -> p h w", h=HPC)
            nc.vector.tensor_copy(
                v4[:, :, HD:HD + 1],
                onesv[:].rearrange("p (a b) -> p a b", b=1),
            )
        vt_sb = [qkv.tile([128, S], F32R, tag=f"vt{hp}", name=f"vtt{hp}") for hp in range(2)]

        # ---- Q.T / K.T projections: (d'=hp*128 partitions, s free) ----
        qt_sb = [qkv.tile([128, S], F32R, tag=f"qt{hp}", name=f"qtt{hp}") for hp in range(2)]
        kt_sb = [qkv.tile([128, S], F32R, tag=f"kt{hp}", name=f"ktt{hp}") for hp in range(2)]
        with tc.tile_pool(name="ps_p", bufs=1, space="PSUM") as ps_p:
            for which, xin_dram, w_sb, b_sb, dst in (
                ("v", xv, wv_sb, bv_sb, vt_sb),
                ("q", xq, wq_sb, bq_sb, qt_sb),
                ("k", xk, wk_sb, bk_sb, kt_sb),
            ):
                accs = {}
                for hp in range(2):
                    for pc in range(4):
                        accs[(hp, pc)] = ps_p.tile([128, 512], F32, tag=f"pp{hp * 4 + pc}", name=f"pp_{which}{hp}{pc}")
                for d in range(DCH):
                    xt = xin.tile([128, S], F32R, tag="x")
                    nc.sync.dma_start(xt[:], xin_dram[d * 128:(d + 1) * 128, :])
                    for hp in range(2):
                        for pc in range(4):
                            nc.tensor.matmul(
                                accs[(hp, pc)][:],
                                w_sb[d][:, hp * 128:(hp + 1) * 128],
                                xt[:, pc * 512:(pc + 1) * 512],
                                start=(d == 0), stop=(d == DCH - 1),
                            )
                for hp in range(2):
                    for pc in range(4):
                        nc.vector.tensor_scalar_add(
                            dst[hp][:, pc * 512:(pc + 1) * 512],
                            accs[(hp, pc)][:],
                            b_sb[hp][:],
                        )
            # V.T -> V transposes last: dense PE work (~8us) bridging the
            # proj->attention boundary while K/Q evacuations drain, so the
            # PE clock stays un-throttled into the attention phase
            for hp in range(2):
                for st in range(QT):
                    tp = ps_p.tile([128, 128], F32R, tag=f"pp{st % 8}",
                                   name=f"tp{hp}{st}")
                    nc.tensor.transpose(
                        tp[:],
                        vt_sb[hp][:, st * 128:(st + 1) * 128],
                        id_sb[:],
                    )
                    v4 = v_sb[st][:].rearrange("p (h w) -> p h w", h=HPC)
                    nc.vector.tensor_copy(
                        v4[:, 2 * hp:2 * hp + 2, 0:HD],
                        tp[:].rearrange("p (h w) -> p h w", h=2),
                    )

        # ---- attention + normalization, head-pairs packed on PE rows ----
        otn_sb = [qkv.tile([128, S], F32R, tag=f"otn{j}", name=f"otn{j}") for j in range(2)]
        with tc.tile_pool(name="ps_s", bufs=1, space="PSUM") as ps_s, \
             tc.tile_pool(name="ps_o", bufs=1, space="PSUM") as ps_o:

            def emit_outproj(qc):
                # out-proj for a finished q-chunk; emitted during the NEXT
                # chunk's attention so its matmuls fill PE slack there
                for qt_i in range(qc * QCW // 128, (qc + 1) * QCW // 128):
                    ysb = yp.tile([128, D], F32, tag="y", name=f"ysb{qt_i}")
                    for dc in range(2):
                        yps = ps_o.tile([128, 512], F32, tag=f"ot{dc}",
                                        name=f"yps{qt_i}{dc}")
                        for g in range(2):
                            nc.tensor.matmul(
                                yps[:],
                                otn_sb[g][:, qt_i * 128:(qt_i + 1) * 128],
                                wo_sb[g][:, dc * 512:(dc + 1) * 512],
                                start=(g == 0), stop=(g == 1),
                            )
                        nc.vector.tensor_copy(ysb[:, dc * 512:(dc + 1) * 512],
                                              yps[:])
                    nc.sync.dma_start(y[qt_i * 128:(qt_i + 1) * 128, :], ysb[:])

            pending = []
            for qc in range(QCN):
                for j in range(2):          # head pair: heads 2j, 2j+1
                    if j == 1 and pending:
                        emit_outproj(pending.pop())
                    ot_ps = [ps_o.tile([HD + 1, QCW], F32, tag=f"ot{h2}", name=f"ot{qc}{j}{h2}")
                             for h2 in range(2)]
                    pts = {}
                    for kt in range(QT):
                        s_ps = [None, None]
                        for h2 in range(2):  # h2=0 -> rows 0:64, h2=1 -> 64:128
                            h = 2 * j + h2
                            s_ps[h2] = ps_s.tile([128, QCW], F32, tag=f"s{h2}", name=f"sps{h2}")
                            for half in range(2):
                                nc.tensor.matmul(
                                    s_ps[h2][:, half * 512:(half + 1) * 512],
                                    kt_sb[j][h2 * 64:h2 * 64 + 64,
                                             kt * 128:(kt + 1) * 128],
                                    qt_sb[j][h2 * 64:h2 * 64 + 64,
                                             qc * QCW + half * 512:
                                             qc * QCW + (half + 1) * 512],
                                    start=True, stop=True,
                                    tile_position=(h2 * 64, 0),
                                )
                        for h2 in range(2):
                            h = 2 * j + h2
                            pt = ptp.tile([128, QCW], ADT, tag=f"pt{h2}")
                            nc.scalar.activation(
                                pt[:], s_ps[h2][:],
                                mybir.ActivationFunctionType.Exp,
                                scale=1.0 / SCALE,
                            )
                            for half in range(2):
                                nc.tensor.matmul(
                                    ot_ps[h2][:, half * 512:(half + 1) * 512],
                                    v_sb[kt][:, h * 65:h * 65 + 65],
                                    pt[:, half * 512:(half + 1) * 512],
                                    start=(kt == 0), stop=(kt == QT - 1),
                                )
                    # evacuate O.T+sums to SBUF fast (frees the psum slot
                    # for the next head pair), then normalize off-path
                    for h2 in range(2):
                        h = 2 * j + h2
                        otr = nrm.tile([HD + 1, QCW], F32, tag="otr")
                        nc.vector.tensor_copy(otr[:], ot_ps[h2][:])
                        rc32 = nrm.tile([1, QCW], F32, tag="rc32")
                        nc.vector.reciprocal(rc32[:], otr[HD:HD + 1, :])
                        sc = nrm.tile([HD, QCW], F32, tag="sc")
                        nc.gpsimd.partition_broadcast(sc[:], rc32[:])
                        if h2 == 0:
                            nc.vector.tensor_mul(
                                otn_sb[j][0:HD, qc * QCW:(qc + 1) * QCW],
                                otr[0:HD, :], sc[:],
                            )
                        else:
                            stg = nrm.tile([HD, QCW], F32R, tag="stg")
                            nc.vector.tensor_mul(stg[:], otr[0:HD, :], sc[:])
                            nc.sync.dma_start(
                                otn_sb[j][HD:2 * HD, qc * QCW:(qc + 1) * QCW],
                                stg[:],
                            )
                pending.append(qc)
            emit_outproj(pending.pop())

    nc.compile()
    return nc


_NC_CACHE = None


def _get_nc():
    global _NC_CACHE
    if _NC_CACHE is None:
        _NC_CACHE = build_nc()
    return _NC_CACHE


def shard_inputs(query, key, value, Wq, bq, Wk, bk, Wv, bv, Wo, bo):
    """Build the 8 per-core input maps (host-side shard + transpose)."""
    import ml_dtypes
    f = np.float32
    bf = ml_dtypes.bfloat16
    in_maps = []
    for c in range(NCORES):
        b = c // 4
        g = c % 4
        hs = slice(g * DPC, (g + 1) * DPC)
        in_maps.append({
            "xq_t": np.ascontiguousarray(np.asarray(query[b], f).T),
            "xk_t": np.ascontiguousarray(np.asarray(key[b], f).T),
            "xv_t": np.ascontiguousarray(np.asarray(value[b], f).T),
            "wq_t": np.ascontiguousarray(np.asarray(Wq[hs, :], f).T),
            "wk_t": np.ascontiguousarray(np.asarray(Wk[hs, :], f).T),
            "wv_t": np.ascontiguousarray(np.asarray(Wv[hs, :], f).T),
            "wo_t": np.ascontiguousarray(np.asarray(Wo[:, hs], f).T),
            "bq": np.asarray(bq[hs], f).reshape(DPC, 1).copy(),
            "bk": np.asarray(bk[hs], f).reshape(DPC, 1).copy(),
            "bv": np.asarray(bv[hs], f).reshape(DPC, 1).copy(),
            "ident": np.eye(128, dtype=f),
        })
    return in_maps


def kernel(query, key, value, Wq, bq, Wk, bk, Wv, bv, Wo, bo, **run_kwargs):
    nc = _get_nc()
    in_maps = shard_inputs(query, key, value, Wq, bq, Wk, bk, Wv, bv, Wo, bo)
    res = run_bass_kernel_spmd(nc, in_maps, core_ids=list(range(NCORES)),
                               **run_kwargs)
    out = np.zeros((B, S, D), np.float32)
    for c in range(NCORES):
        out[c // 4] += res.results[c]["y"]
    out += np.asarray(bo, np.float32)
    if run_kwargs:
        kernel.last_result = res
    return out



# revision 3
# speedup vs baseline: 1.1006x; 1.1006x over previous
"""Multi-head attention (B=2, S=2048, D=1024, H=16) on 8 NeuronCores.

Sharding: core c -> batch c//4, head-group c%4 (4 heads, 256 proj dims).
Per-core Bass/Tile kernel computes Q/K/V projections, transposed-scores
attention (k on partitions, softmax without max-subtraction), and a
partial output projection (row-parallel Wo). Host sums the 4 partials
per batch and adds bo.

All matmuls run as float32r (TF32-like, 1 cycle/row) with fp32 PSUM
accumulation; data is shipped as fp32 bits (float32r is bit-compatible).
"""

import sys

sys.path.insert(0, "/opt/trn_rl_repo")

from contextlib import ExitStack

import numpy as np

import concourse.bacc as bacc
import concourse.mybir as mybir
import concourse.tile as tile
from concourse.bass_utils import run_bass_kernel_spmd

B = 2
S = 2048
D = 1024
H = 16
HD = 64
HPC = 4          # heads per core
DPC = HPC * HD   # 256 projection dims per core
NCORES = 8
SCALE = 8.0      # sqrt(HD)

F32 = mybir.dt.float32
F32R = mybir.dt.float32r
BF16 = mybir.dt.bfloat16
ADT = BF16   # attention operand dtype (qt/kt/v/pt)

DCH = D // 128   # 8 contraction chunks of 128
QT = S // 128    # 16 q-tiles / k-tiles of 128
QCN = 2          # attention q-chunks of 1024
QCW = 1024


def build_nc():
    nc = bacc.Bacc("TRN2", target_bir_lowering=False, debug=False, num_devices=NCORES)

    xq = nc.dram_tensor("xq_t", [D, S], BF16, kind="ExternalInput")
    xk = nc.dram_tensor("xk_t", [D, S], BF16, kind="ExternalInput")
    xv = nc.dram_tensor("xv_t", [D, S], BF16, kind="ExternalInput")
    wq = nc.dram_tensor("wq_t", [D, DPC], BF16, kind="ExternalInput")
    wk = nc.dram_tensor("wk_t", [D, DPC], BF16, kind="ExternalInput")
    wv = nc.dram_tensor("wv_t", [D, DPC], BF16, kind="ExternalInput")
    wo = nc.dram_tensor("wo_t", [DPC, D], BF16, kind="ExternalInput")
    bq = nc.dram_tensor("bq", [DPC, 1], F32, kind="ExternalInput")
    bk = nc.dram_tensor("bk", [DPC, 1], F32, kind="ExternalInput")
    bv = nc.dram_tensor("bv", [DPC, 1], F32, kind="ExternalInput")
    ident = nc.dram_tensor("ident", [128, 128], BF16, kind="ExternalInput")
    y = nc.dram_tensor("y", [S, D], F32, kind="ExternalOutput")

    with tile.TileContext(nc) as tc, ExitStack() as ctx:
        const = ctx.enter_context(tc.tile_pool(name="const", bufs=1))
        xin = ctx.enter_context(tc.tile_pool(name="xin", bufs=5))
        qkv = ctx.enter_context(tc.tile_pool(name="qkv", bufs=1))
        ptp = ctx.enter_context(tc.tile_pool(name="ptp", bufs=2))
        nrm = ctx.enter_context(tc.tile_pool(name="nrm", bufs=2))
        yp = ctx.enter_context(tc.tile_pool(name="yp", bufs=3))
        # tag semantics: same tag -> rotate through `bufs` slots;
        # distinct tags -> independent allocations.

        # ---- constants / weights ----
        # tiny dummy exp first: preloads the ACT exp table set off the
        # critical path (a ~4us PE-idle gap at attention start re-throttles
        # the PE clock to 1.2GHz for the rest of the kernel otherwise)
        dmy = const.tile([1, 16], F32, tag="dmy")
        nc.vector.memset(dmy[:], 0.0)
        dmy2 = const.tile([1, 16], F32, tag="dmy2")
        nc.scalar.activation(dmy2[:], dmy[:], mybir.ActivationFunctionType.Exp)

        # memset can't target f32r; stage in f32 and round via DVE copy
        ones32 = const.tile([1, 128], F32, tag="ones32")
        nc.vector.memset(ones32[:], 1.0)
        ones = const.tile([1, 128], F32R, tag="ones")
        nc.vector.tensor_copy(ones[:], ones32[:])
        onesv32 = const.tile([128, HPC], F32, tag="onesv32")
        nc.vector.memset(onesv32[:], 1.0)
        onesv = const.tile([128, HPC], F32R, tag="onesv")
        nc.vector.tensor_copy(onesv[:], onesv32[:])
        wq_sb = [const.tile([128, DPC], BF16, tag=f"wq{d}", name=f"wq{d}") for d in range(DCH)]
        wk_sb = [const.tile([128, DPC], BF16, tag=f"wk{d}", name=f"wk{d}") for d in range(DCH)]
        wv_sb = [const.tile([128, DPC], BF16, tag=f"wv{d}", name=f"wv{d}") for d in range(DCH)]
        for d in range(DCH):
            nc.sync.dma_start(wq_sb[d][:], wq[d * 128:(d + 1) * 128, :])
            nc.sync.dma_start(wk_sb[d][:], wk[d * 128:(d + 1) * 128, :])
            nc.sync.dma_start(wv_sb[d][:], wv[d * 128:(d + 1) * 128, :])
        wo_sb = [const.tile([128, D], BF16, tag=f"wo{g}", name=f"wo{g}") for g in range(2)]
        for g in range(2):
            nc.sync.dma_start(wo_sb[g][:], wo[g * 128:(g + 1) * 128, :])
        bq_sb = [const.tile([128, 1], F32, tag=f"bq{hp}", name=f"bq{hp}") for hp in range(2)]
        bk_sb = [const.tile([128, 1], F32, tag=f"bk{hp}", name=f"bk{hp}") for hp in range(2)]
        bv_sb = [const.tile([128, 1], F32, tag=f"bv{hp}", name=f"bv{hp}") for hp in range(2)]
        for hp in range(2):
            nc.sync.dma_start(bq_sb[hp][:], bq[hp * 128:(hp + 1) * 128, :])
            nc.sync.dma_start(bk_sb[hp][:], bk[hp * 128:(hp + 1) * 128, :])
            nc.sync.dma_start(bv_sb[hp][:], bv[hp * 128:(hp + 1) * 128, :])
        id_sb = const.tile([128, 128], BF16, tag="id")
        nc.sync.dma_start(id_sb[:], ident[:])

        # ---- V tiles (128, 4*65) with ones column, filled by PE transpose
        # of a V.T projection (weight-stationary like Q/K; avoids the
        # per-matmul LDWEIGHTS serialization of an x-stationary V-proj) ----
        v_sb = [qkv.tile([128, HPC * (HD + 1)], ADT, tag=f"v{st}", name=f"v{st}") for st in range(QT)]
        for st in range(QT):
            v4 = v_sb[st][:].rearrange("p (h w) -> p h w", h=HPC)
            nc.vector.tensor_copy(
                v4[:, :, HD:HD + 1],
                onesv[:].rearrange("p (a b) -> p a b", b=1),
            )
        vt_sb = [qkv.tile([128, S], BF16, tag=f"vt{hp}", name=f"vtt{hp}") for hp in range(2)]

        # ---- Q.T / K.T projections: (d'=hp*128 partitions, s free) ----
        qt_sb = [qkv.tile([128, S], ADT, tag=f"qt{hp}", name=f"qtt{hp}") for hp in range(2)]
        kt_sb = [qkv.tile([128, S], ADT, tag=f"kt{hp}", name=f"ktt{hp}") for hp in range(2)]
        with tc.tile_pool(name="ps_p", bufs=1, space="PSUM") as ps_p:
            for which, xin_dram, w_sb, b_sb, dst in (
                ("v", xv, wv_sb, bv_sb, vt_sb),
                ("q", xq, wq_sb, bq_sb, qt_sb),
                ("k", xk, wk_sb, bk_sb, kt_sb),
            ):
                accs = {}
                for hp in range(2):
                    for pc in range(4):
                        accs[(hp, pc)] = ps_p.tile([128, 512], F32, tag=f"pp{hp * 4 + pc}", name=f"pp_{which}{hp}{pc}")
                for d in range(DCH):
                    xt = xin.tile([128, S], BF16, tag="x")
                    nc.sync.dma_start(xt[:], xin_dram[d * 128:(d + 1) * 128, :])
                    for hp in range(2):
                        for pc in range(4):
                            nc.tensor.matmul(
                                accs[(hp, pc)][:],
                                w_sb[d][:, hp * 128:(hp + 1) * 128],
                                xt[:, pc * 512:(pc + 1) * 512],
                                start=(d == 0), stop=(d == DCH - 1),
                            )
                for hp in range(2):
                    for pc in range(4):
                        nc.vector.tensor_scalar_add(
                            dst[hp][:, pc * 512:(pc + 1) * 512],
                            accs[(hp, pc)][:],
                            b_sb[hp][:],
                        )
            # V.T -> V transposes last: dense PE work (~8us) bridging the
            # proj->attention boundary while K/Q evacuations drain, so the
            # PE clock stays un-throttled into the attention phase
            for hp in range(2):
                for st in range(QT):
                    tp = ps_p.tile([128, 128], BF16, tag=f"pp{st % 8}",
                                   name=f"tp{hp}{st}")
                    nc.tensor.transpose(
                        tp[:],
                        vt_sb[hp][:, st * 128:(st + 1) * 128],
                        id_sb[:],
                    )
                    v4 = v_sb[st][:].rearrange("p (h w) -> p h w", h=HPC)
                    nc.vector.tensor_copy(
                        v4[:, 2 * hp:2 * hp + 2, 0:HD],
                        tp[:].rearrange("p (h w) -> p h w", h=2),
                    )

        # ---- attention + normalization, head-pairs packed on PE rows ----
        otn_sb = [qkv.tile([128, S], BF16, tag=f"otn{j}", name=f"otn{j}") for j in range(2)]
        with tc.tile_pool(name="ps_s", bufs=1, space="PSUM") as ps_s, \
             tc.tile_pool(name="ps_o", bufs=1, space="PSUM") as ps_o:

            def emit_outproj(qc):
                # out-proj for a finished q-chunk; emitted during the NEXT
                # chunk's attention so its matmuls fill PE slack there
                for qt_i in range(qc * QCW // 128, (qc + 1) * QCW // 128):
                    ysb = yp.tile([128, D], F32, tag="y", name=f"ysb{qt_i}")
                    for dc in range(2):
                        yps = ps_o.tile([128, 512], F32, tag=f"ot{dc}",
                                        name=f"yps{qt_i}{dc}")
                        for g in range(2):
                            nc.tensor.matmul(
                                yps[:],
                                otn_sb[g][:, qt_i * 128:(qt_i + 1) * 128],
                                wo_sb[g][:, dc * 512:(dc + 1) * 512],
                                start=(g == 0), stop=(g == 1),
                            )
                        nc.vector.tensor_copy(ysb[:, dc * 512:(dc + 1) * 512],
                                              yps[:])
                    nc.sync.dma_start(y[qt_i * 128:(qt_i + 1) * 128, :], ysb[:])

            pending = []
            for qc in range(QCN):
                for j in range(2):          # head pair: heads 2j, 2j+1
                    if j == 1 and pending:
                        emit_outproj(pending.pop())
                    ot_ps = [ps_o.tile([HD + 1, QCW], F32, tag=f"ot{h2}", name=f"ot{qc}{j}{h2}")
                             for h2 in range(2)]
                    pts = {}
                    for kt in range(QT):
                        s_ps = [None, None]
                        for h2 in range(2):  # h2=0 -> rows 0:64, h2=1 -> 64:128
                            h = 2 * j + h2
                            s_ps[h2] = ps_s.tile([128, QCW], F32, tag=f"s{h2}", name=f"sps{h2}")
                            for half in range(2):
                                nc.tensor.matmul(
                                    s_ps[h2][:, half * 512:(half + 1) * 512],
                                    kt_sb[j][h2 * 64:h2 * 64 + 64,
                                             kt * 128:(kt + 1) * 128],
                                    qt_sb[j][h2 * 64:h2 * 64 + 64,
                                             qc * QCW + half * 512:
                                             qc * QCW + (half + 1) * 512],
                                    start=True, stop=True,
                                    tile_position=(h2 * 64, 0),
                                )
                        for h2 in range(2):
                            h = 2 * j + h2
                            pt = ptp.tile([128, QCW], ADT, tag=f"pt{h2}")
                            nc.scalar.activation(
                                pt[:], s_ps[h2][:],
                                mybir.ActivationFunctionType.Exp,
                                scale=1.0 / SCALE,
                            )
                            for half in range(2):
                                nc.tensor.matmul(
                                    ot_ps[h2][:, half * 512:(half + 1) * 512],
                                    v_sb[kt][:, h * 65:h * 65 + 65],
                                    pt[:, half * 512:(half + 1) * 512],
                                    start=(kt == 0), stop=(kt == QT - 1),
                                )
                    # evacuate O.T+sums to SBUF fast (frees the psum slot
                    # for the next head pair), then normalize off-path
                    for h2 in range(2):
                        h = 2 * j + h2
                        otr = nrm.tile([HD + 1, QCW], F32, tag="otr")
                        nc.vector.tensor_copy(otr[:], ot_ps[h2][:])
                        rc32 = nrm.tile([1, QCW], F32, tag="rc32")
                        nc.vector.reciprocal(rc32[:], otr[HD:HD + 1, :])
                        sc = nrm.tile([HD, QCW], F32, tag="sc")
                        nc.gpsimd.partition_broadcast(sc[:], rc32[:])
                        if h2 == 0:
                            nc.vector.tensor_mul(
                                otn_sb[j][0:HD, qc * QCW:(qc + 1) * QCW],
                                otr[0:HD, :], sc[:],
                            )
                        else:
                            stg = nrm.tile([HD, QCW], BF16, tag="stg")
                            nc.vector.tensor_mul(stg[:], otr[0:HD, :], sc[:])
                            nc.sync.dma_start(
                                otn_sb[j][HD:2 * HD, qc * QCW:(qc + 1) * QCW],
                                stg[:],
                            )
                pending.append(qc)
            emit_outproj(pending.pop())

    nc.compile()
    return nc


_NC_CACHE = None


def _get_nc():
    global _NC_CACHE
    if _NC_CACHE is None:
        _NC_CACHE = build_nc()
    return _NC_CACHE


def shard_inputs(query, key, value, Wq, bq, Wk, bk, Wv, bv, Wo, bo):
    """Build the 8 per-core input maps (host-side shard + transpose)."""
    import ml_dtypes
    f = np.float32
    bf = ml_dtypes.bfloat16
    in_maps = []
    for c in range(NCORES):
        b = c // 4
        g = c % 4
        hs = slice(g * DPC, (g + 1) * DPC)
        in_maps.append({
            "xq_t": np.ascontiguousarray(np.asarray(query[b], f).T).astype(bf),
            "xk_t": np.ascontiguousarray(np.asarray(key[b], f).T).astype(bf),
            "xv_t": np.ascontiguousarray(np.asarray(value[b], f).T).astype(bf),
            "wq_t": np.ascontiguousarray(np.asarray(Wq[hs, :], f).T).astype(bf),
            "wk_t": np.ascontiguousarray(np.asarray(Wk[hs, :], f).T).astype(bf),
            "wv_t": np.ascontiguousarray(np.asarray(Wv[hs, :], f).T).astype(bf),
            "wo_t": np.ascontiguousarray(np.asarray(Wo[:, hs], f).T).astype(bf),
            "bq": np.asarray(bq[hs], f).reshape(DPC, 1).copy(),
            "bk": np.asarray(bk[hs], f).reshape(DPC, 1).copy(),
            "bv": np.asarray(bv[hs], f).reshape(DPC, 1).copy(),
            "ident": np.eye(128, dtype=f).astype(bf),
        })
    return in_maps


def kernel(query, key, value, Wq, bq, Wk, bk, Wv, bv, Wo, bo, **run_kwargs):
    nc = _get_nc()
    in_maps = shard_inputs(query, key, value, Wq, bq, Wk, bk, Wv, bv, Wo, bo)
    res = run_bass_kernel_spmd(nc, in_maps, core_ids=list(range(NCORES)),
                               **run_kwargs)
    out = np.zeros((B, S, D), np.float32)
    for c in range(NCORES):
        out[c // 4] += res.results[c]["y"]
    out += np.asarray(bo, np.float32)
    if run_kwargs:
        kernel.last_result = res
    return out



# revision 6
# speedup vs baseline: 1.1446x; 1.0400x over previous
"""Multi-head attention (B=2, S=2048, D=1024, H=16) on 8 NeuronCores.

Sharding: core c -> batch c//4, head-group c%4 (4 heads, 256 proj dims).
Per-core Bass/Tile kernel computes Q/K/V projections, transposed-scores
attention (k on partitions, softmax without max-subtraction), and a
partial output projection (row-parallel Wo). Host sums the 4 partials
per batch and adds bo.

All matmuls run as float32r (TF32-like, 1 cycle/row) with fp32 PSUM
accumulation; data is shipped as fp32 bits (float32r is bit-compatible).
"""

import sys

sys.path.insert(0, "/opt/trn_rl_repo")

from contextlib import ExitStack

import numpy as np

import concourse.bacc as bacc
import concourse.mybir as mybir
import concourse.tile as tile
from concourse.bass_utils import run_bass_kernel_spmd

B = 2
S = 2048
D = 1024
H = 16
HD = 64
HPC = 4          # heads per core
DPC = HPC * HD   # 256 projection dims per core
NCORES = 8
SCALE = 8.0      # sqrt(HD)

F32 = mybir.dt.float32
F32R = mybir.dt.float32r
BF16 = mybir.dt.bfloat16
ADT = BF16   # attention operand dtype (qt/kt/v/pt)

DCH = D // 128   # 8 contraction chunks of 128
QT = S // 128    # 16 q-tiles / k-tiles of 128
QCN = 2          # attention q-chunks of 1024
QCW = 1024


def build_nc():
    nc = bacc.Bacc("TRN2", target_bir_lowering=False, debug=False, num_devices=NCORES)

    xq = nc.dram_tensor("xq_t", [D, S], BF16, kind="ExternalInput")
    xk = nc.dram_tensor("xk_t", [D, S], BF16, kind="ExternalInput")
    xv = nc.dram_tensor("xv_t", [D, S], BF16, kind="ExternalInput")
    wq = nc.dram_tensor("wq_t", [D, DPC], BF16, kind="ExternalInput")
    wk = nc.dram_tensor("wk_t", [D, DPC], BF16, kind="ExternalInput")
    wv = nc.dram_tensor("wv_t", [D, DPC], BF16, kind="ExternalInput")
    wo = nc.dram_tensor("wo_t", [DPC, D], BF16, kind="ExternalInput")
    bq = nc.dram_tensor("bq", [DPC, 1], F32, kind="ExternalInput")
    bk = nc.dram_tensor("bk", [DPC, 1], F32, kind="ExternalInput")
    bv = nc.dram_tensor("bv", [DPC, 1], F32, kind="ExternalInput")
    ident = nc.dram_tensor("ident", [128, 128], BF16, kind="ExternalInput")
    y = nc.dram_tensor("y", [S, D], BF16, kind="ExternalOutput")

    with tile.TileContext(nc) as tc, ExitStack() as ctx:
        const = ctx.enter_context(tc.tile_pool(name="const", bufs=1))
        xin = ctx.enter_context(tc.tile_pool(name="xin", bufs=5))
        qkv = ctx.enter_context(tc.tile_pool(name="qkv", bufs=1))
        yp = ctx.enter_context(tc.tile_pool(name="yp", bufs=3))
        ptp = ctx.enter_context(tc.tile_pool(name="ptp", bufs=2))
        nrm = ctx.enter_context(tc.tile_pool(name="nrm", bufs=2))
        # tag semantics: same tag -> rotate through `bufs` slots;
        # distinct tags -> independent allocations.

        # ---- constants / weights ----
        # tiny dummy exp first: preloads the ACT exp table set off the
        # critical path (a ~4us PE-idle gap at attention start re-throttles
        # the PE clock to 1.2GHz for the rest of the kernel otherwise)
        dmy = const.tile([1, 16], F32, tag="dmy")
        nc.vector.memset(dmy[:], 0.0)
        dmy2 = const.tile([1, 16], F32, tag="dmy2")
        nc.scalar.activation(dmy2[:], dmy[:], mybir.ActivationFunctionType.Exp)

        # memset can't target f32r; stage in f32 and round via DVE copy
        ones32 = const.tile([1, 128], F32, tag="ones32")
        nc.vector.memset(ones32[:], 1.0)
        ones = const.tile([1, 128], F32R, tag="ones")
        nc.vector.tensor_copy(ones[:], ones32[:])
        onesv32 = const.tile([128, HPC], F32, tag="onesv32")
        nc.vector.memset(onesv32[:], 1.0)
        onesv = const.tile([128, HPC], F32R, tag="onesv")
        nc.vector.tensor_copy(onesv[:], onesv32[:])
        wq_sb = [const.tile([128, DPC], BF16, tag=f"wq{d}", name=f"wq{d}") for d in range(DCH)]
        wk_sb = [const.tile([128, DPC], BF16, tag=f"wk{d}", name=f"wk{d}") for d in range(DCH)]
        wv_sb = [const.tile([128, DPC], BF16, tag=f"wv{d}", name=f"wv{d}") for d in range(DCH)]
        for d in range(DCH):
            nc.sync.dma_start(wq_sb[d][:], wq[d * 128:(d + 1) * 128, :])
            nc.sync.dma_start(wk_sb[d][:], wk[d * 128:(d + 1) * 128, :])
            nc.sync.dma_start(wv_sb[d][:], wv[d * 128:(d + 1) * 128, :])
        wo_sb = [const.tile([128, D], BF16, tag=f"wo{g}", name=f"wo{g}") for g in range(2)]
        for g in range(2):
            nc.sync.dma_start(wo_sb[g][:], wo[g * 128:(g + 1) * 128, :])
        bq_sb = [const.tile([128, 1], F32, tag=f"bq{hp}", name=f"bq{hp}") for hp in range(2)]
        bk_sb = [const.tile([128, 1], F32, tag=f"bk{hp}", name=f"bk{hp}") for hp in range(2)]
        bv_sb = [const.tile([128, 1], F32, tag=f"bv{hp}", name=f"bv{hp}") for hp in range(2)]
        for hp in range(2):
            nc.sync.dma_start(bq_sb[hp][:], bq[hp * 128:(hp + 1) * 128, :])
            nc.sync.dma_start(bk_sb[hp][:], bk[hp * 128:(hp + 1) * 128, :])
            nc.sync.dma_start(bv_sb[hp][:], bv[hp * 128:(hp + 1) * 128, :])
        id_sb = const.tile([128, 128], BF16, tag="id")
        nc.sync.dma_start(id_sb[:], ident[:])

        # ---- V tiles (128, 4*65) with ones column, filled by PE transpose
        # of a V.T projection (weight-stationary like Q/K; avoids the
        # per-matmul LDWEIGHTS serialization of an x-stationary V-proj) ----
        v_sb = [qkv.tile([128, HPC * (HD + 1)], ADT, tag=f"v{st}", name=f"v{st}") for st in range(QT)]
        for st in range(QT):
            v4 = v_sb[st][:].rearrange("p (h w) -> p h w", h=HPC)
            nc.vector.tensor_copy(
                v4[:, :, HD:HD + 1],
                onesv[:].rearrange("p (a b) -> p a b", b=1),
            )
        vt_sb = [qkv.tile([128, S], BF16, tag=f"vt{hp}", name=f"vtt{hp}") for hp in range(2)]

        # ---- Q.T / K.T projections: (d'=hp*128 partitions, s free) ----
        qt_sb = [qkv.tile([128, S], ADT, tag=f"qt{hp}", name=f"qtt{hp}") for hp in range(2)]
        kt_sb = [qkv.tile([128, S], ADT, tag=f"kt{hp}", name=f"ktt{hp}") for hp in range(2)]
        with tc.tile_pool(name="ps_p", bufs=1, space="PSUM") as ps_p:
            for which, xin_dram, w_sb, b_sb, dst in (
                ("v", xv, wv_sb, bv_sb, vt_sb),
                ("q", xq, wq_sb, bq_sb, qt_sb),
                ("k", xk, wk_sb, bk_sb, kt_sb),
            ):
                accs = {}
                for hp in range(2):
                    for pc in range(4):
                        accs[(hp, pc)] = ps_p.tile([128, 512], F32, tag=f"pp{hp * 4 + pc}", name=f"pp_{which}{hp}{pc}")
                for d in range(DCH):
                    xt = xin.tile([128, S], BF16, tag="x")
                    nc.sync.dma_start(xt[:], xin_dram[d * 128:(d + 1) * 128, :])
                    for hp in range(2):
                        for pc in range(4):
                            nc.tensor.matmul(
                                accs[(hp, pc)][:],
                                w_sb[d][:, hp * 128:(hp + 1) * 128],
                                xt[:, pc * 512:(pc + 1) * 512],
                                start=(d == 0), stop=(d == DCH - 1),
                            )
                for hp in range(2):
                    for pc in range(4):
                        nc.vector.tensor_scalar_add(
                            dst[hp][:, pc * 512:(pc + 1) * 512],
                            accs[(hp, pc)][:],
                            b_sb[hp][:],
                        )
            # V.T -> V transposes last: dense PE work (~8us) bridging the
            # proj->attention boundary while K/Q evacuations drain, so the
            # PE clock stays un-throttled into the attention phase
            for hp in range(2):
                for st in range(QT):
                    tp = ps_p.tile([128, 128], BF16, tag=f"pp{st % 8}",
                                   name=f"tp{hp}{st}")
                    nc.tensor.transpose(
                        tp[:],
                        vt_sb[hp][:, st * 128:(st + 1) * 128],
                        id_sb[:],
                    )
                    v4 = v_sb[st][:].rearrange("p (h w) -> p h w", h=HPC)
                    nc.vector.tensor_copy(
                        v4[:, 2 * hp:2 * hp + 2, 0:HD],
                        tp[:].rearrange("p (h w) -> p h w", h=2),
                    )

        # ---- attention + normalization, head-pairs packed on PE rows ----
        otn_sb = [qkv.tile([128, S], BF16, tag=f"otn{j}", name=f"otn{j}") for j in range(2)]
        with tc.tile_pool(name="ps_s", bufs=1, space="PSUM") as ps_s, \
             tc.tile_pool(name="ps_o", bufs=1, space="PSUM") as ps_o:

            def emit_outproj(qc):
                # out-proj for a finished q-chunk; emitted during the NEXT
                # chunk's attention so its matmuls fill PE slack there
                for qt_i in range(qc * QCW // 128, (qc + 1) * QCW // 128):
                    ysb = yp.tile([128, D], BF16, tag="y", name=f"ysb{qt_i}")
                    for dc in range(2):
                        yps = ps_o.tile([128, 512], F32, tag=f"ot{dc}",
                                        name=f"yps{qt_i}{dc}")
                        for g in range(2):
                            nc.tensor.matmul(
                                yps[:],
                                otn_sb[g][:, qt_i * 128:(qt_i + 1) * 128],
                                wo_sb[g][:, dc * 512:(dc + 1) * 512],
                                start=(g == 0), stop=(g == 1),
                            )
                        nc.scalar.activation(
                            ysb[:, dc * 512:(dc + 1) * 512], yps[:],
                            mybir.ActivationFunctionType.Copy,
                        )
                    nc.sync.dma_start(y[qt_i * 128:(qt_i + 1) * 128, :], ysb[:])

            pending = []
            for qc in range(QCN):
                for j in range(2):          # head pair: heads 2j, 2j+1
                    if j == 1 and pending:
                        emit_outproj(pending.pop())
                    ot_ps = [ps_o.tile([HD + 1, QCW], F32, tag=f"ot{h2}", name=f"ot{qc}{j}{h2}")
                             for h2 in range(2)]
                    pts = {}
                    for kt in range(QT):
                        s_ps = [None, None]
                        for h2 in range(2):  # h2=0 -> rows 0:64, h2=1 -> 64:128
                            h = 2 * j + h2
                            s_ps[h2] = ps_s.tile([128, QCW], F32, tag=f"s{h2}", name=f"sps{h2}")
                            for half in range(2):
                                nc.tensor.matmul(
                                    s_ps[h2][:, half * 512:(half + 1) * 512],
                                    kt_sb[j][h2 * 64:h2 * 64 + 64,
                                             kt * 128:(kt + 1) * 128],
                                    qt_sb[j][h2 * 64:h2 * 64 + 64,
                                             qc * QCW + half * 512:
                                             qc * QCW + (half + 1) * 512],
                                    start=True, stop=True,
                                    tile_position=(h2 * 64, 0),
                                )
                        for h2 in range(2):
                            h = 2 * j + h2
                            pt = ptp.tile([128, QCW], ADT, tag=f"pt{h2}")
                            nc.scalar.activation(
                                pt[:], s_ps[h2][:],
                                mybir.ActivationFunctionType.Exp,
                                scale=1.0 / SCALE,
                            )
                            for half in range(2):
                                nc.tensor.matmul(
                                    ot_ps[h2][:, half * 512:(half + 1) * 512],
                                    v_sb[kt][:, h * 65:h * 65 + 65],
                                    pt[:, half * 512:(half + 1) * 512],
                                    start=(kt == 0), stop=(kt == QT - 1),
                                )
                    # evacuate O.T+sums to SBUF fast (frees the psum slot
                    # for the next head pair), then normalize off-path
                    for h2 in range(2):
                        h = 2 * j + h2
                        otr = nrm.tile([HD + 1, QCW], BF16, tag="otr")
                        nc.vector.tensor_copy(otr[:], ot_ps[h2][:])
                        rc32 = nrm.tile([1, QCW], BF16, tag="rc32")
                        with nc.allow_low_precision(reason="bf16 softmax denom; tol 2e-2"):
                            nc.vector.reciprocal(rc32[:], otr[HD:HD + 1, :])
                        sc = nrm.tile([HD, QCW], BF16, tag="sc")
                        nc.gpsimd.partition_broadcast(sc[:], rc32[:])
                        nc.vector.tensor_mul(
                            otn_sb[j][h2 * HD:(h2 + 1) * HD,
                                      qc * QCW:(qc + 1) * QCW],
                            otr[0:HD, :], sc[:],
                        )
                pending.append(qc)
            emit_outproj(pending.pop())

    nc.compile()
    return nc


_NC_CACHE = None


def _get_nc():
    global _NC_CACHE
    if _NC_CACHE is None:
        _NC_CACHE = build_nc()
    return _NC_CACHE


def shard_inputs(query, key, value, Wq, bq, Wk, bk, Wv, bv, Wo, bo):
    """Build the 8 per-core input maps (host-side shard + transpose)."""
    import ml_dtypes
    f = np.float32
    bf = ml_dtypes.bfloat16
    in_maps = []
    for c in range(NCORES):
        b = c // 4
        g = c % 4
        hs = slice(g * DPC, (g + 1) * DPC)
        in_maps.append({
            "xq_t": np.ascontiguousarray(np.asarray(query[b], f).T).astype(bf),
            "xk_t": np.ascontiguousarray(np.asarray(key[b], f).T).astype(bf),
            "xv_t": np.ascontiguousarray(np.asarray(value[b], f).T).astype(bf),
            "wq_t": np.ascontiguousarray(np.asarray(Wq[hs, :], f).T).astype(bf),
            "wk_t": np.ascontiguousarray(np.asarray(Wk[hs, :], f).T).astype(bf),
            "wv_t": np.ascontiguousarray(np.asarray(Wv[hs, :], f).T).astype(bf),
            "wo_t": np.ascontiguousarray(np.asarray(Wo[:, hs], f).T).astype(bf),
            "bq": np.asarray(bq[hs], f).reshape(DPC, 1).copy(),
            "bk": np.asarray(bk[hs], f).reshape(DPC, 1).copy(),
            "bv": np.asarray(bv[hs], f).reshape(DPC, 1).copy(),
            "ident": np.eye(128, dtype=f).astype(bf),
        })
    return in_maps


def kernel(query, key, value, Wq, bq, Wk, bk, Wv, bv, Wo, bo, **run_kwargs):
    nc = _get_nc()
    in_maps = shard_inputs(query, key, value, Wq, bq, Wk, bk, Wv, bv, Wo, bo)
    res = run_bass_kernel_spmd(nc, in_maps, core_ids=list(range(NCORES)),
                               **run_kwargs)
    out = np.zeros((B, S, D), np.float32)
    for c in range(NCORES):
        out[c // 4] += np.asarray(res.results[c]["y"], np.float32)
    out += np.asarray(bo, np.float32)
    if run_kwargs:
        kernel.last_result = res
    return out

